# revision 64
# baseline (speedup 1.0000x reference)
"""Multi-head attention Bass kernel for Trainium2 (8 NeuronCores).

Problem: B=2, N=4096, E=768, H=12 heads of dim 64 (nn_MultiHeadAttention).
Sharding: 2 batches x 4 head-groups (3 heads each) = 8 cores.

Per-core pipeline (fp8 DoubleRow edition):
  - QKV projection in bf16 (x and w_qkv cast to bf16 on host; 1 cyc/row).
  - Q stored as fp8e4 (hi, lo) pair: hi = e4m3(psum), lo = e4m3(psum+bq-hi),
    so hi+lo = q+bq to ~fp8^2 precision and the Q bias rides in lo.
  - K stored as fp8e4 duplicated across the two DoubleRow k-subtiles.
  - Scores: one fp8 DoubleRow matmul per kv tile: (K,K) x (Qhi,Qlo) ->
    s = (q+bq).K at 0.5 cyc/row.
  - exp: ACT Exp(scale=1/8) psum->fp8 directly, or (pool route, fraction PF)
    per-kv DVE copies into one SBUF tile + ONE Pool pow((e^1/8)^s) -> fp8.
  - PV: one fp8 DoubleRow matmul per kv PAIR: lhsT = V tile [128, 2, 128]
    (64 v-dims + 64 ones cols), rhs = p pair tile. The ones BLOCK replicates
    the softmax denominator across psum partitions 64:128 (free: matmul cost
    is output-free-size driven), so normalization is a full-lane [64,512]
    reciprocal + mul on DVE with no gpsimd partition_broadcast.
  - Output projection in f32r against w_proj rows -> partial [N, 768];
    psum->SBUF copies land in one [128, 768] tile per row-block so each
    row-block is a single out-DMA.
Host: sums the 4 partials per batch and adds the (bias-folded) b_proj.

PSUM (8 banks): "sc" ring = 2 x 2-bank tiles (stage-A psq/psv + ACT-route
scores); "scp" ring = 2 x 1-bank pool-route score tiles; "pv" ring =
2 x 1-bank tiles shared by PV accumulators (head-major) and out-proj.

DMA: stage-A x chunks load as one [128, KE, 512] transfer; the wqk K-slice
is dispatched first (it gates the first score tile), wpT after chunk 1.

Bias handling (exact algebra): K bias drops out of softmax; V bias commutes
through normalization and folds into b_proj (host); Q bias folded into Q-lo.

Knob notes (TimelineSim-swept): SPILL (interleaving prologue jobs inside
stage-A emission) and TAIL_ACT consistently hurt; NORM="pool" (gpsimd
divide) relieves DVE busy but lengthens the yn latency chain; a fused DVE
divide is illegal on TRN2 (one-PSUM-operand rule, NCC_IBVF027).
"""

import sys

sys.path.insert(0, "/opt/trn_rl_repo")

import numpy as np
import ml_dtypes

import concourse.bass as bass  # noqa: E402
import concourse.mybir as mybir  # noqa: E402
import concourse.tile as tile  # noqa: E402
from concourse import bacc  # noqa: E402
from concourse.bass_utils import run_bass_kernel_spmd  # noqa: E402

F32 = mybir.dt.float32
F32R = mybir.dt.float32r
BF16 = mybir.dt.bfloat16
FP8 = mybir.dt.float8e4
AF = mybir.ActivationFunctionType
ALU = mybir.AluOpType
DRMODE = mybir.MatmulPerfMode.DoubleRow

B, N, E = 2, 4096, 768
H, HD = 12, 64
NH = 3          # heads per core
M_GROUPS = 4    # head groups (tensor parallel)
VW = 128        # PV lhsT width: 64 v-dims + 64 ones cols (denominator block)

# tuning knobs
KVQ = 2         # kv tiles per score-psum tile (both routes)
SC_BUFS = 2     # score ring depth (2 banks each)
PF = 0.38       # steady-state pool-route fraction (evenly spread)
PRO_ACT = 0     # during prologue, every PRO_ACT-th job takes the ACT route
AHEAD = 5       # score tiles emitted ahead of the consuming exp
PVD = 4         # PV consumption delayed this many jobs behind exp emission
OST_ACT_EVERY = 4  # every Nth out-proj copy on ACT (0 = all DVE)
NORM = "recip"  # "recip" | "pool" (copy + gpsimd divide)
PP_TAG = "pv"   # psum ring for out-proj tiles: "pv" | "scp"
XT_MERGE = True   # one DMA per stage-A x chunk (vs per-k tiles)
SPILL = False     # interleave prologue jobs inside stage-A emission
POW_BATCH = True  # single pow over both kv halves (vs per-kv)
PVD_LAST = 4      # PV delay for the last-kvq job of each (qg, h)
M2FULL = True   # full-lane m2 staging (q2/k2 via one ACT copy + DMAs)
OST_BATCH = True  # batch out-proj DMA per (qg, qb) row-block of 768
TAIL_ACT = 0      # last N jobs forced onto the ACT exp route
QLO = True      # Q as fp8 (hi, lo) pair; False = plain fp8 dup + ACT bias
XT_SPLIT0 = False # split chunk-0 x DMA so m1 starts earlier
PF_EARLY = None   # pool fraction for the first PF_E0 steady jobs (None = PF)
PF_E0 = 96
QDUP = "dma"      # q-dup engine when QLO=False: "dma" | "pool"
# WARNING: route-dependent PV delays reorder same-(qg,h) PV accumulation
# (start/stop flags) and CORRUPT results unless kvq order is enforced.
PVD_ACT = None    # PV delay override for ACT-route jobs (None = PVD)
PVD_POOL = None   # PV delay override for pool-route jobs (None = PVD)


def build_nc(n_tokens=N, num_devices=8):
    """Build the per-core Bass module (SPMD: same program, different data)."""
    n = n_tokens
    NQG = n // 512          # q groups of 512
    NKV = n // 128          # kv blocks of 128
    KE = E // 128           # contraction tiles over E

    nc = bacc.Bacc("TRN2", target_bir_lowering=False, debug=False,
                   num_devices=num_devices)

    xT = nc.dram_tensor("xT", [E, n], BF16, kind="ExternalInput")
    wqkT = nc.dram_tensor("wqkT", [E, 3 * 128], BF16, kind="ExternalInput")
    wvT = nc.dram_tensor("wvT", [E, NH * HD], BF16, kind="ExternalInput")
    bq = nc.dram_tensor("bq", [2, 128], F32, kind="ExternalInput")
    wpT = nc.dram_tensor("wpT", [HD, NH, E], F32R, kind="ExternalInput")
    cpow = nc.dram_tensor("cpow", [128, 1], F32, kind="ExternalInput")
    out = nc.dram_tensor("out", [n, E], F32, kind="ExternalOutput")

    with tile.TileContext(nc) as tc:
        with (
            tc.tile_pool(name="perm", bufs=1) as perm,
            tc.tile_pool(name="wpool", bufs=1) as wpool,
        ):
            # Persistent SBUF tensors.
            q_sb = perm.tile([128, 2, n], FP8, name="q_sb")
            k_sb = perm.tile([128, 2, n], FP8, name="k_sb")
            q2_sb = perm.tile([64, 2, n], FP8, name="q2_sb")
            k2_sb = perm.tile([64, 2, n], FP8, name="k2_sb")
            # V in [kv, d] layout: per kv-block of 128 tokens,
            # 3 heads x (64 dims + 64 ones cols for the denominator block).
            v_sb = perm.tile([128, NKV, NH, VW], FP8, name="v_sb")

            wqkT_sb = wpool.tile([128, KE, 3 * 128], BF16, name="wqk_sb")
            wvT_sb = wpool.tile([128, KE, NH * HD], BF16, name="wv_sb")
            wpT_sb = wpool.tile([64, NH, E], F32R, name="wp_sb")
            bq_sb = wpool.tile([128, 2], F32, name="bq_sb")
            cp_sb = wpool.tile([128, 1], F32, name="cp_sb")

            # K-weight slice first: it gates the first score tile. wvT and
            # the remaining wqk slices are dispatched inside stage-A chunk 0
            # after its x tiles; wpT (out-proj, needed last) goes after the
            # first prologue jobs.
            def wqk_slice(m):
                nc.sync.dma_start(
                    wqkT_sb[:, :, m * 128:(m + 1) * 128],
                    wqkT[:, m * 128:(m + 1) * 128].rearrange(
                        "(a p) c -> p a c", p=128))

            wqk_slice(1)

            # ones block for the softmax-denominator trick: psum rows 64:128
            # of every PV accumulation replicate the denominator.
            nc.gpsimd.memset(v_sb[:, :, :, HD:VW], 1.0)

            with (
                tc.tile_pool(name="scpsum", bufs=1, space="PSUM") as scpsum,
                tc.tile_pool(name="pvpsum", bufs=1, space="PSUM") as pvpsum,
                tc.tile_pool(name="xpool", bufs=18) as xpool,
                tc.tile_pool(name="spool", bufs=3) as spool,
            ):
                # PE p-state warmup: tiny matmuls on garbage SBUF into a
                # dummy psum tile during the initial DMA wait.
                scratch = wpool.tile([64, 256], FP8, name="warm_src")
                nc.gpsimd.memset(scratch[:], 0.0)
                warm = pvpsum.tile([128, 64], F32, tag="pv", bufs=2,
                                   name="warmup")
                for wi in range(40):
                    nc.tensor.matmul(warm[:], scratch[:, 0:128],
                                     scratch[:, 0:64],
                                     start=(wi == 0), stop=(wi == 39))

                # ---- Stage A: QKV projection (bf16), quantize to fp8 ----
                def emit_stageA(ng, spill=None):
                    # spill: callback run between m-phases to interleave
                    # prologue-job emission (spreads DVE/Pool feed evenly)
                    if XT_MERGE:
                        xt = xpool.tile([128, KE, 512], BF16, tag="xt",
                                        bufs=2, name=f"xt{ng}")
                        qsd = slice(ng * 512, (ng + 1) * 512)
                        if ng == 0 and XT_SPLIT0:
                            # chunk 0 gates everything: land the first two
                            # k-tiles early so m1 can start sooner
                            nc.sync.dma_start(
                                xt[:, 0:2, :], xT[0:256, qsd].rearrange(
                                    "(a p) c -> p a c", p=128))
                            nc.sync.dma_start(
                                xt[:, 2:KE, :], xT[256:E, qsd].rearrange(
                                    "(a p) c -> p a c", p=128))
                        else:
                            nc.sync.dma_start(
                                xt[:], xT[:, qsd].rearrange(
                                    "(a p) c -> p a c", p=128))
                        xsl = [(xt, k) for k in range(KE)]
                    else:
                        xsl = []
                        for k in range(KE):
                            xtk = xpool.tile([128, 1, 512], BF16, tag="xt",
                                             bufs=12, name=f"xt{ng}_{k}")
                            nc.sync.dma_start(
                                xtk[:, 0, :], xT[k * 128:(k + 1) * 128,
                                                 ng * 512:(ng + 1) * 512])
                            xsl.append((xtk, 0))
                    if ng == 0:
                        wqk_slice(0)
                        nc.sync.dma_start(bq_sb[:], bq.rearrange("a p -> p a"))
                        nc.sync.dma_start(
                            wvT_sb[:],
                            wvT.rearrange("(a p) c -> p a c", p=128))
                        wqk_slice(2)
                        nc.sync.dma_start(cp_sb[:], cpow[:])
                    qs = slice(ng * 512, (ng + 1) * 512)
                    for m in (1, 0, 2):
                        psq = scpsum.tile([128, 512], F32, tag="sc",
                                          bufs=SC_BUFS, name=f"psq{ng}_{m}")
                        for k in range(KE):
                            xtk, ki = xsl[k]
                            nc.tensor.matmul(psq[:],
                                             wqkT_sb[:, k, m * 128:(m + 1) * 128],
                                             xtk[:, ki, :], start=(k == 0),
                                             stop=(k == KE - 1))
                        if m == 0:      # Q heads 0,1
                            if QLO:
                                nc.scalar.copy(q_sb[:, 0, qs], psq[:])
                                nc.vector.scalar_tensor_tensor(
                                    q_sb[:, 1, qs], psq[:], bq_sb[:, 0:1],
                                    q_sb[:, 0, qs], op0=ALU.add,
                                    op1=ALU.subtract)
                            else:
                                # plain fp8 Q with the bias folded into the
                                # ACT copy; dup for the DR subtile pair
                                nc.scalar.activation(q_sb[:, 0, qs], psq[:],
                                                     AF.Identity,
                                                     bias=bq_sb[:, 0:1])
                                if ng == 0:
                                    nc.vector.tensor_copy(q_sb[:, 1, qs],
                                                          q_sb[:, 0, qs])
                                elif QDUP == "pool":
                                    nc.gpsimd.tensor_copy(q_sb[:, 1, qs],
                                                          q_sb[:, 0, qs])
                                else:
                                    nc.sync.dma_start(q_sb[:, 1, qs],
                                                      q_sb[:, 0, qs])
                        elif m == 1:    # K heads 0,1 (+ dup)
                            nc.scalar.copy(k_sb[:, 0, qs], psq[:])
                            if ng == 0:
                                nc.vector.tensor_copy(k_sb[:, 1, qs],
                                                      k_sb[:, 0, qs])
                            else:
                                nc.sync.dma_start(k_sb[:, 1, qs],
                                                  k_sb[:, 0, qs])
                        elif M2FULL:    # m2 = [Q2 ; K2], full-lane staging
                            t2 = xpool.tile([128, 512], FP8, tag="k2st",
                                            bufs=3, name=f"t2_{ng}")
                            if QLO:
                                nc.scalar.copy(t2[:], psq[:])
                                nc.vector.scalar_tensor_tensor(
                                    q2_sb[:, 1, qs], psq[0:64, :],
                                    bq_sb[0:64, 1:2], t2[0:64, :],
                                    op0=ALU.add, op1=ALU.subtract)
                                nc.sync.dma_start(q2_sb[:, 0, qs], t2[0:64, :])
                            else:
                                # bq col 1 is zero on partitions 64:128, so
                                # the K2 rows pass through unbiased
                                nc.scalar.activation(t2[:], psq[:],
                                                     AF.Identity,
                                                     bias=bq_sb[:, 1:2])
                                nc.sync.dma_start(q2_sb[:, 0, qs], t2[0:64, :])
                                nc.sync.dma_start(q2_sb[:, 1, qs], t2[0:64, :])
                            nc.sync.dma_start(k2_sb[:, 0, qs], t2[64:128, :])
                            nc.sync.dma_start(k2_sb[:, 1, qs], t2[64:128, :])
                        else:
                            nc.scalar.copy(q2_sb[:, 0, qs], psq[0:64, :])
                            nc.vector.scalar_tensor_tensor(
                                q2_sb[:, 1, qs], psq[0:64, :], bq_sb[0:64, 1:2],
                                q2_sb[:, 0, qs], op0=ALU.add, op1=ALU.subtract)
                            k2st = xpool.tile([128, 512], FP8, tag="k2st",
                                              bufs=3, name=f"k2st{ng}")
                            nc.vector.tensor_copy(k2st[64:128, :],
                                                  psq[64:128, :])
                            nc.sync.dma_start(k2_sb[:, 0, qs], k2st[64:128, :])
                            nc.sync.dma_start(k2_sb[:, 1, qs], k2st[64:128, :])
                        if spill:
                            spill()
                    # V projection (bf16): 2 kv-blocks per psum tile
                    for vj in range(2):
                        psv = scpsum.tile([128, 2, NH * HD], F32, tag="sc",
                                          bufs=SC_BUFS, name=f"psv{ng}_{vj}")
                        for j in range(2):
                            jj = 2 * vj + j
                            for k in range(KE):
                                xtk, ki = xsl[k]
                                nc.tensor.matmul(
                                    psv[:, j, :],
                                    xtk[:, ki, jj * 128:(jj + 1) * 128],
                                    wvT_sb[:, k, :], start=(k == 0),
                                    stop=(k == KE - 1))
                        kvt = ng * 4 + 2 * vj
                        nc.scalar.copy(
                            v_sb[:, kvt:kvt + 2, :, 0:HD],
                            psv.rearrange("p a (h c) -> p a h c", c=HD))
                        if spill:
                            spill()

                # ---- Stage B+C: software-pipelined attention (head-major) --
                HEADS = {0: (q_sb, k_sb, 0), 1: (q_sb, k_sb, 64),
                         2: (q2_sb, k2_sb, 0)}
                NQD = NKV // KVQ   # score tiles per (qg, h)
                jobs = [(0, h, kvq) for kvq in range(NQD) for h in (0, 1)]
                jobs += [(0, 2, kvq) for kvq in range(NQD)]
                jobs += [(qg, h, kvq) for qg in range(1, NQG)
                         for h in range(NH) for kvq in range(NQD)]
                NJOBS = len(jobs)
                pvp_tiles = {}
                yn = {}
                ost_tiles = {}

                def emit_scores(qg, h, kvq, rt=0):
                    # rt: 0 = ACT route on the sc ring; 1 = pool route on the
                    # scp ring; 2 = ACT route on the scp ring (prologue: the
                    # sc ring is occupied by stage-A psq/psv tiles)
                    qsl = slice(qg * 512, (qg + 1) * 512)
                    qt, kt, pb = HEADS[h]
                    if rt:
                        tiles = []
                        for j in range(KVQ):
                            kv = kvq * KVQ + j
                            scj = scpsum.tile([128, 512], F32, tag="scp",
                                              bufs=2,
                                              name=f"scp{qg}_{h}_{kv}")
                            if h == 2:
                                lhs = kt[:, :, kv * 128:(kv + 1) * 128]
                                rhs = qt[:, :, qsl]
                            else:
                                lhs = kt[pb:pb + 64, :,
                                         kv * 128:(kv + 1) * 128]
                                rhs = qt[pb:pb + 64, :, qsl]
                            nc.tensor.matmul(scj[:], lhs, rhs, start=True,
                                             stop=True, perf_mode=DRMODE)
                            tiles.append(scj)
                        return tiles
                    sc = scpsum.tile([128, KVQ, 512], F32, tag="sc",
                                     bufs=SC_BUFS, name=f"sc{qg}_{h}_{kvq}")
                    for j in range(KVQ):
                        kv = kvq * KVQ + j
                        if h == 2:
                            lhs = kt[:, :, kv * 128:(kv + 1) * 128]
                            rhs = qt[:, :, qsl]
                        else:
                            lhs = kt[pb:pb + 64, :, kv * 128:(kv + 1) * 128]
                            rhs = qt[pb:pb + 64, :, qsl]
                        nc.tensor.matmul(sc[:, j, :], lhs, rhs, start=True,
                                         stop=True, perf_mode=DRMODE)
                    return sc

                def emit_norm(qg, h):
                    pv = pvp_tiles[(qg, h)]
                    ynt = spool.tile([64, 512], F32R, tag="yn", bufs=8,
                                     name=f"yn{qg}_{h}")
                    if NORM == "pool":
                        # one DVE copy of pv+denominator, divide on Pool
                        pvs = spool.tile([128, 512], F32, tag="rcp", bufs=3,
                                         name=f"pvs{qg}_{h}")
                        nc.vector.tensor_copy(pvs[:], pv[:])
                        nc.gpsimd.tensor_tensor(
                            ynt[:], pvs[0:HD, :], pvs[HD:2 * HD, :],
                            op=ALU.divide)
                    else:
                        rcp = spool.tile([64, 512], F32, tag="rcp", bufs=3,
                                         name=f"rcp{qg}_{h}")
                        nc.vector.reciprocal(rcp[:], pv[HD:2 * HD, :])
                        nc.vector.tensor_mul(ynt[:], pv[0:HD, :], rcp[:])
                    yn[(qg, h)] = ynt

                def proj_thunks(qg):
                    thunks = []
                    ost_i = 0
                    FS = ((0, 512), (512, 256))
                    order = ([(qb, f) for qb in range(4) for f in range(2)]
                             if OST_BATCH else
                             [(qb, f) for f in range(2) for qb in range(4)])
                    for qb, f in order:
                        ost_i += 1
                        f0, fw = FS[f]

                        def blk(qg=qg, f=f, qb=qb, f0=f0, fw=fw,
                                on_act=(OST_ACT_EVERY > 0
                                        and ost_i % OST_ACT_EVERY == 0)):
                            pp = (pvpsum if PP_TAG == "pv" else scpsum).tile(
                                [128, fw], F32, tag=PP_TAG, bufs=2,
                                name=f"pp{qg}_{f}_{qb}")
                            for h in range(NH):
                                nc.tensor.matmul(
                                    pp[:],
                                    yn[(qg, h)][:, qb * 128:(qb + 1) * 128],
                                    wpT_sb[:, h, f0:f0 + fw],
                                    start=(h == 0), stop=(h == NH - 1))
                            cpy = nc.scalar.copy if on_act else \
                                nc.vector.tensor_copy
                            rows = slice(qg * 512 + qb * 128,
                                         qg * 512 + (qb + 1) * 128)
                            if OST_BATCH:
                                if f == 0:
                                    ost_tiles[(qg, qb)] = spool.tile(
                                        [128, E], F32, tag="ost", bufs=6,
                                        name=f"ost{qg}_{qb}")
                                ostt = ost_tiles[(qg, qb)]
                                cpy(ostt[:, f0:f0 + fw], pp[:])
                                if f == 1:
                                    nc.sync.dma_start(out[rows, :], ostt[:])
                            else:
                                ostt = spool.tile([128, fw], F32, tag="ost",
                                                  bufs=8,
                                                  name=f"ost{qg}_{f}_{qb}")
                                cpy(ostt[:], pp[:])
                                nc.sync.dma_start(out[rows, f0:f0 + fw],
                                                  ostt[:])
                        thunks.append(blk)
                    return thunks

                laggard = []   # delayed PV thunks: (emit_at_idx, key, fn)
                pv_next = {}   # (qg, hh) -> next kvq allowed to emit its PV

                def flush_laggards(now):
                    # PV matmuls of one (qg, hh) accumulation group MUST hit
                    # the PE stream in kvq order (start=True resets psum) —
                    # a PV entry fires only when it is the group's next kvq.
                    progress = True
                    while progress:
                        progress = False
                        i = 0
                        while i < len(laggard):
                            at, key, fn = laggard[i]
                            ok = at <= now and (
                                key is None
                                or pv_next.get(key[:2], 0) == key[2])
                            if ok:
                                laggard.pop(i)
                                if key is not None:
                                    pv_next[key[:2]] = key[2] + 1
                                fn()
                                progress = True
                            else:
                                i += 1

                def do_job(idx, qg, hh, kvq, sc, rt):
                    if kvq == 0:
                        pvp_tiles[(qg, hh)] = pvpsum.tile(
                            [VW, 512], F32, tag="pv", bufs=2,
                            name=f"pv{qg}_{hh}")
                    p_tile = spool.tile([128, KVQ, 512], FP8, tag="p",
                                        bufs=max(PVD, PVD_POOL or 0) + 5,
                                        name=f"p{qg}_{hh}_{kvq}")
                    if rt == 1:
                        # per-kv DVE copies into one SBUF tile + ONE pow
                        scs = spool.tile([128, KVQ, 512], F32, tag="scs",
                                         bufs=5, name=f"scs{qg}_{hh}_{kvq}")
                        for j, scj in enumerate(sc):
                            nc.vector.tensor_copy(scs[:, j, :], scj[:])
                        if POW_BATCH:
                            nc.gpsimd.tensor_tensor(
                                p_tile[:, :, :],
                                cp_sb[:, 0:1].broadcast_to([128, KVQ, 512]),
                                scs[:, :, :], op=ALU.pow)
                        else:
                            for j in range(KVQ):
                                nc.gpsimd.tensor_tensor(
                                    p_tile[:, j, :],
                                    cp_sb[:, 0:1].broadcast_to([128, 512]),
                                    scs[:, j, :], op=ALU.pow)
                    elif rt == 2:
                        for j, scj in enumerate(sc):
                            nc.scalar.activation(p_tile[:, j, :], scj[:],
                                                 AF.Exp, scale=0.125)
                    else:
                        nc.scalar.activation(p_tile[:], sc[:], AF.Exp,
                                             scale=0.125 if QLO else 0.0625)

                    def mk_pv(qg=qg, hh=hh, kvq=kvq, p_tile=p_tile):
                        def pv():
                            for t in range(KVQ // 2):
                                kv = kvq * KVQ + 2 * t
                                nc.tensor.matmul(
                                    pvp_tiles[(qg, hh)],
                                    v_sb[:, kv:kv + 2, hh, :],
                                    p_tile[:, 2 * t:2 * t + 2, :],
                                    start=(kv == 0), stop=(kv == NKV - 2),
                                    perf_mode=DRMODE)
                            if kvq == NQD - 1:
                                emit_norm(qg, hh)
                                if hh == 2:
                                    for di, blk in enumerate(proj_thunks(qg)):
                                        laggard.append(
                                            (min(idx + PVD + 1 + di,
                                                 NJOBS - 1), None, blk))
                        return pv

                    base_pvd = PVD
                    if rt == 1 and PVD_POOL is not None:
                        base_pvd = PVD_POOL
                    elif rt != 1 and PVD_ACT is not None:
                        base_pvd = PVD_ACT
                    pvd = base_pvd if idx < NJOBS - 2 * PVD else 1
                    if kvq == NQD - 1:
                        pvd = min(pvd, PVD_LAST)
                    laggard.append((min(idx + pvd, NJOBS - 1), (qg, hh, kvq),
                                    mk_pv()))
                    flush_laggards(idx)

                def route(i):
                    if i < 2 * NQD:
                        if PRO_ACT and (i % PRO_ACT) == PRO_ACT - 1:
                            return 2
                        return 1
                    if i >= NJOBS - TAIL_ACT:
                        return 0   # tail: single-op ACT exp, shortest chain
                    f = PF_EARLY if (PF_EARLY is not None and i < PF_E0) \
                        else PF
                    return 1 if int(i * f) != int((i - 1) * f) else 0

                emitted = 0
                JPG = 2 * NQD // NQG   # prologue jobs per stage-A chunk

                def mk_spill(budget):
                    state = {"left": budget}

                    def spill():
                        nonlocal emitted
                        if state["left"] <= 0 or emitted >= len(jobs):
                            return
                        state["left"] -= 1
                        qg, hh, kvq = jobs[emitted]
                        r = route(emitted)
                        sc = emit_scores(qg, hh, kvq, r)
                        do_job(emitted, qg, hh, kvq, sc, r)
                        emitted += 1
                    return spill

                for ng in range(NQG):
                    sp = mk_spill(JPG) if SPILL else None
                    emit_stageA(ng, sp)
                    if ng == 1:
                        nc.sync.dma_start(wpT_sb[:], wpT[:])
                    while emitted < (ng + 1) * JPG:
                        qg, hh, kvq = jobs[emitted]
                        r = route(emitted)
                        sc = emit_scores(qg, hh, kvq, r)
                        do_job(emitted, qg, hh, kvq, sc, r)
                        emitted += 1
                # steady state with score lookahead
                pending = [emit_scores(*jobs[emitted + i], route(emitted + i))
                           for i in range(AHEAD)]
                for idx in range(emitted, len(jobs)):
                    qg, hh, kvq = jobs[idx]
                    sc = pending.pop(0)
                    if idx + AHEAD < len(jobs):
                        pending.append(emit_scores(*jobs[idx + AHEAD],
                                                   route(idx + AHEAD)))
                    do_job(idx, qg, hh, kvq, sc, route(idx))
                flush_laggards(10 ** 9)

    nc.finalize()
    return nc


def host_prep(x, w_qkv, b_qkv, w_proj, b_proj, n_tokens=N):
    """Build per-core input maps + the host-side combine closure."""
    x = np.asarray(x, np.float32)
    w_qkv = np.asarray(w_qkv, np.float32)
    b_qkv = np.asarray(b_qkv, np.float32)
    w_proj = np.asarray(w_proj, np.float32)
    b_proj = np.asarray(b_proj, np.float32)

    xT = [np.ascontiguousarray(x[b].T).astype(ml_dtypes.bfloat16)
          for b in range(B)]  # [E, N] bf16

    in_maps = []
    for c in range(8):
        b, g = divmod(c, M_GROUPS)
        base = g * NH * 3 * HD  # row offset of this group in w_qkv (576/group)
        wq = [w_qkv[base + i * 3 * HD: base + i * 3 * HD + HD] for i in range(NH)]
        wk = [w_qkv[base + i * 3 * HD + HD: base + i * 3 * HD + 2 * HD]
              for i in range(NH)]
        wv = [w_qkv[base + i * 3 * HD + 2 * HD: base + i * 3 * HD + 3 * HD]
              for i in range(NH)]
        bqv = [b_qkv[base + i * 3 * HD: base + i * 3 * HD + HD] for i in range(NH)]
        # m-tiles: m0=[Q0;Q1], m1=[K0;K1], m2=[Q2;K2]
        wqkT = np.concatenate(
            [wq[0], wq[1], wk[0], wk[1], wq[2], wk[2]], axis=0).T  # [E, 384]
        wvT = np.concatenate(wv, axis=0).T  # [E, 192]
        bq = np.zeros((2, 128), np.float32)
        bq[0, 0:HD] = bqv[0]
        bq[0, HD:2 * HD] = bqv[1]
        bq[1, 0:HD] = bqv[2]
        # wpT[d, h, f] = w_proj[f, g*192 + h*64 + d]
        wp = w_proj[:, g * NH * HD:(g + 1) * NH * HD]  # [768, 192]
        wpT = np.ascontiguousarray(
            wp.T.reshape(NH, HD, E).transpose(1, 0, 2))  # [64, 3, 768]
        in_maps.append({
            "xT": xT[b],
            "wqkT": np.ascontiguousarray(wqkT).astype(ml_dtypes.bfloat16),
            "wvT": np.ascontiguousarray(wvT).astype(ml_dtypes.bfloat16),
            "bq": bq,
            "wpT": wpT,
            "cpow": np.full((128, 1), np.exp(0.125 if QLO else 0.0625), np.float32),
        })

    # fold V bias through the projection into the output bias
    bv_all = np.concatenate(
        [b_qkv[h * 3 * HD + 2 * HD: (h + 1) * 3 * HD] for h in range(H)])  # [768]
    b_eff = b_proj + w_proj @ bv_all

    def combine(results):
        out = np.empty((B, n_tokens, E), np.float32)
        for b in range(B):
            acc = results[b * M_GROUPS]["out"].astype(np.float32)
            for g in range(1, M_GROUPS):
                acc = acc + results[b * M_GROUPS + g]["out"]
            out[b] = acc + b_eff
        return out

    return in_maps, combine


_NC_CACHE = {}


def kernel(x, w_qkv, b_qkv, w_proj, b_proj):
    if "nc" not in _NC_CACHE:
        _NC_CACHE["nc"] = build_nc()
    nc = _NC_CACHE["nc"]
    in_maps, combine = host_prep(x, w_qkv, b_qkv, w_proj, b_proj)
    res = run_bass_kernel_spmd(nc, in_maps, core_ids=list(range(8)))
    return combine(res.results)


if __name__ == "__main__":
    rng = np.random.default_rng(0)
    inputs = {
        "x": rng.normal(size=(B, N, E)).astype(np.float32),
        "w_qkv": (rng.normal(size=(3 * E, E)) * 0.02).astype(np.float32),
        "b_qkv": (rng.normal(size=(3 * E,)) * 0.02).astype(np.float32),
        "w_proj": (rng.normal(size=(E, E)) * 0.02).astype(np.float32),
        "b_proj": (rng.normal(size=(E,)) * 0.02).astype(np.float32),
    }
    out = kernel(**inputs)
    print("out", out.shape, out.dtype, float(np.abs(out).mean()))
